# revision 7
# baseline (speedup 1.0000x reference)
"""Trainium2 Bass kernel for nn_GAT_Model (4x GATConv + global_add_pool + MLP).

Strategy (8 NeuronCores, SPMD single program):
  - Nodes partitioned into 8 equal ranges (1250/core, padded to 1280 local
    slots = 10 dst-blocks of 128).  Edges (incl. self-loops) are assigned to
    the core owning their dst node, sorted by dst, grouped into dst-blocks
    with a uniform tile count T_blk per block (host-padded; pad edges carry
    dstloc=-1 so their selector rows are all-zero and they contribute
    nothing).
  - Per layer: each core matmuls its own node rows producing
    [h@W | al_src | al_dst] (attention coefficients folded into the matmul
    via device-computed W@[A_s|A_d]), AllGathers the [hW | al_src] table,
    then processes its edges: dma_gather of source rows, per-edge softmax
    weights exp(leakyrelu(al_s[src]+al_d[dst])) (logits are O(1) here so no
    max subtraction is needed), aggregation of numerator and denominator via
    one-hot selector matmuls accumulating in PSUM.
  - Pooling via 256-graph selector matmuls -> partial sums -> AllReduce.
  - MLP head sharded by graph (32 graphs/core), final scatter by row ids.

kernel(**inputs) takes FULL inputs and returns the FULL [256,1] output.
"""

import math

import numpy as np

P = 128
NCORES = 8
CHUNK = 1024            # edges per dma_gather call
SLOTS = CHUNK // P      # gather slots (tiles of 128 edges) per chunk


# ---------------------------------------------------------------------------
# Host-side preprocessing (integer index manipulation only)
# ---------------------------------------------------------------------------

def _prep(inputs):
    x = np.asarray(inputs["x"], np.float32)
    edge_index = np.asarray(inputs["edge_index"], np.int64)
    batch = np.asarray(inputs["batch"], np.int64)
    desc = np.asarray(inputs["desc"], np.float32)

    N = x.shape[0]
    B = desc.shape[0]
    npc = N // NCORES
    assert npc * NCORES == N
    nblk = (npc + P - 1) // P
    locpad = nblk * P
    trows = NCORES * locpad
    gpc = B // NCORES
    assert gpc * NCORES == B

    convs = []
    for i in range(1, 5):
        W = np.asarray(inputs[f"W{i}"], np.float32)
        a_s = np.asarray(inputs[f"as{i}"], np.float32)
        a_d = np.asarray(inputs[f"ad{i}"], np.float32)
        b = np.asarray(inputs[f"b{i}"], np.float32)
        H, C = a_s.shape
        convs.append(dict(W=W, a_s=a_s, a_d=a_d, b=b, H=H, C=C,
                          cin=W.shape[0], hc=H * C))

    loops = np.arange(N, dtype=np.int64)
    src = np.concatenate([np.asarray(edge_index[0]), loops])
    dst = np.concatenate([np.asarray(edge_index[1]), loops])

    core_of = dst // npc
    src_tr = locpad * (src // npc) + (src % npc)   # table row per src node
    order = np.argsort(dst, kind="stable")

    blk_edges = [[[] for _ in range(nblk)] for _ in range(NCORES)]
    for e in order:
        c = int(core_of[e])
        d = int(dst[e]) - c * npc
        blk_edges[c][d // P].append((int(src_tr[e]), d - (d // P) * P))

    t_blk = max((len(bl) + P - 1) // P for cb in blk_edges for bl in cb)
    nt = nblk * t_blk
    nt_pad = ((nt + SLOTS - 1) // SLOTS) * SLOTS
    nchunks = nt_pad // SLOTS
    zrow = locpad - 1                      # pad rows are inert (dstloc=-1)

    eidx = np.full((NCORES, nt_pad * P), zrow, np.int16)
    dstloc = np.full((NCORES, nt_pad, P), -1.0, np.float32)
    for c in range(NCORES):
        for b in range(nblk):
            for j, (s_tr, d_loc) in enumerate(blk_edges[c][b]):
                g = b * t_blk + j // P
                eidx[c, g * P + (j % P)] = s_tr
                dstloc[c, g, j % P] = d_loc
    eidx_w = np.zeros((NCORES, P, nt_pad * P // 16), np.int16)
    for c in range(NCORES):
        w = eidx[c].reshape(-1, 16).T
        eidx_w[c] = np.tile(w, (8, 1))
    dstloc_t = np.ascontiguousarray(dstloc.transpose(0, 2, 1))

    batchloc = np.full((NCORES, P, nblk), -1.0, np.float32)
    for c in range(NCORES):
        bl = batch[c * npc:(c + 1) * npc].astype(np.float32)
        for t in range(nblk):
            seg = bl[t * P:(t + 1) * P]
            batchloc[c, :len(seg), t] = seg

    xs = np.zeros((NCORES, locpad, x.shape[1]), np.float32)
    for c in range(NCORES):
        xs[c, :npc] = x[c * npc:(c + 1) * npc]

    amats = []
    for cv in convs:
        H, C, hc = cv["H"], cv["C"], cv["hc"]
        A = np.zeros((hc, 2 * H), np.float32)
        for h in range(H):
            A[h * C:(h + 1) * C, h] = cv["a_s"][h]
            A[h * C:(h + 1) * C, H + h] = cv["a_d"][h]
        amats.append(A)

    S = dict(
        N=N, B=B, npc=npc, nblk=nblk, locpad=locpad, trows=trows,
        gpc=gpc, ghalves=(B + P - 1) // P, t_blk=t_blk, nt=nt,
        nt_pad=nt_pad, nchunks=nchunks, desc_dim=desc.shape[1],
        xin=x.shape[1],
        convs=tuple(dict(H=c["H"], C=c["C"], cin=c["cin"], hc=c["hc"])
                    for c in convs),
        fcs=tuple((np.asarray(inputs[f"Wf{j}"]).shape[0],
                   np.asarray(inputs[f"Wf{j}"]).shape[1])
                  for j in range(1, 6)),
        wg_in=np.asarray(inputs["Wg"]).shape[0],
        wg_out=np.asarray(inputs["Wg"]).shape[1],
    )
    S["tcols"] = tuple(int(math.ceil((c["hc"] + c["H"]) * 4 / 256) * 64)
                       for c in S["convs"])

    arangerow = np.tile(np.arange(256, dtype=np.float32), (P, 1))
    arangecol = np.arange(P, dtype=np.float32).reshape(P, 1)
    ident = np.eye(P, dtype=np.float32)

    in_maps = []
    for c in range(NCORES):
        m = dict(
            x=xs[c],
            eidx=eidx_w[c],
            dstloc=dstloc_t[c],
            batchloc=batchloc[c],
            gids=(c * gpc + np.arange(gpc, dtype=np.int32)).reshape(gpc, 1),
            desc=np.ascontiguousarray(desc[c * gpc:(c + 1) * gpc]),
            arangerow=arangerow, arangecol=arangecol, ident=ident,
            wg=np.asarray(inputs["Wg"], np.float32),
            bg=np.tile(np.asarray(inputs["bg"], np.float32), (P, 1)),
        )
        for i, cv in enumerate(convs):
            m[f"w{i+1}"] = cv["W"]
            m[f"amat{i+1}"] = amats[i]
            m[f"bias{i+1}"] = np.tile(cv["b"], (P, 1))
        for j in range(1, 6):
            m[f"wf{j}"] = np.asarray(inputs[f"Wf{j}"], np.float32)
            m[f"bf{j}"] = np.tile(np.asarray(inputs[f"bf{j}"], np.float32),
                                  (P, 1))
        in_maps.append(m)
    return S, in_maps


# ---------------------------------------------------------------------------
# Device program
# ---------------------------------------------------------------------------

def _ceil(a, b):
    return (a + b - 1) // b


def _col_chunks(n, lim=512):
    out, c = [], 0
    while c < n:
        out.append((c, min(c + lim, n)))
        c = min(c + lim, n)
    return out


def _build(S, enable_asserts=False):
    import contextlib

    import concourse.bass as bass
    import concourse.mybir as mybir
    import concourse.tile as tile
    from concourse import bacc

    dt = mybir.dt.float32
    nblk, t_blk, nt_pad = S["nblk"], S["t_blk"], S["nt_pad"]
    locpad, trows = S["locpad"], S["trows"]
    gpc, B = S["gpc"], S["B"]
    maxH = max(cv["H"] for cv in S["convs"])

    nc = bacc.Bacc("TRN2", target_bir_lowering=False, debug=False,
                   enable_asserts=enable_asserts, num_devices=NCORES)

    # ---- I/O ----
    t_x = nc.dram_tensor("x", [locpad, S["xin"]], dt, kind="ExternalInput")
    t_eidx = nc.dram_tensor("eidx", [P, nt_pad * P // 16], mybir.dt.int16,
                            kind="ExternalInput")
    t_dstloc = nc.dram_tensor("dstloc", [P, nt_pad], dt, kind="ExternalInput")
    t_batchloc = nc.dram_tensor("batchloc", [P, nblk], dt,
                                kind="ExternalInput")
    t_gids = nc.dram_tensor("gids", [gpc, 1], mybir.dt.int32,
                            kind="ExternalInput")
    t_desc = nc.dram_tensor("desc", [gpc, S["desc_dim"]], dt,
                            kind="ExternalInput")
    t_ar = nc.dram_tensor("arangerow", [P, 256], dt, kind="ExternalInput")
    t_ac = nc.dram_tensor("arangecol", [P, 1], dt, kind="ExternalInput")
    t_id = nc.dram_tensor("ident", [P, P], dt, kind="ExternalInput")
    t_wg = nc.dram_tensor("wg", [S["wg_in"], S["wg_out"]], dt,
                          kind="ExternalInput")
    t_bg = nc.dram_tensor("bg", [P, S["wg_out"]], dt, kind="ExternalInput")
    t_w, t_amat, t_bias, t_wf, t_bf = {}, {}, {}, {}, {}
    for i, cv in enumerate(S["convs"]):
        t_w[i] = nc.dram_tensor(f"w{i+1}", [cv["cin"], cv["hc"]], dt,
                                kind="ExternalInput")
        t_amat[i] = nc.dram_tensor(f"amat{i+1}", [cv["hc"], 2 * cv["H"]], dt,
                                   kind="ExternalInput")
        t_bias[i] = nc.dram_tensor(f"bias{i+1}", [P, cv["hc"]], dt,
                                   kind="ExternalInput")
    for j, (fi, fo) in enumerate(S["fcs"]):
        t_wf[j] = nc.dram_tensor(f"wf{j+1}", [fi, fo], dt,
                                 kind="ExternalInput")
        t_bf[j] = nc.dram_tensor(f"bf{j+1}", [P, fo], dt,
                                 kind="ExternalInput")
    t_out = nc.dram_tensor("out", [B, 1], dt, kind="ExternalOutput")

    # ---- internal DRAM ----
    tc_widths = sorted(set(S["tcols"]))
    t_bounce = {w: nc.dram_tensor(f"bounce{w}", [locpad, w], dt,
                                  kind="Internal") for w in tc_widths}
    t_table = {w: nc.dram_tensor(f"table{w}", [trows, w], dt,
                                 kind="Internal", addr_space="Shared")
               for w in tc_widths}
    maxin = max(cv["cin"] for cv in S["convs"])
    t_hbuf = [nc.dram_tensor(f"hbuf{k}", [locpad, maxin], dt,
                             kind="Internal") for k in range(2)]
    hc4 = S["convs"][3]["hc"]
    t_pin = nc.dram_tensor("pooled_in", [B, hc4], dt, kind="Internal")
    t_pout = nc.dram_tensor("pooled_out", [B, hc4], dt, kind="Internal",
                            addr_space="Shared")

    groups = [list(range(NCORES))]
    Exp = mybir.ActivationFunctionType.Exp
    Copy = mybir.ActivationFunctionType.Copy
    AT = mybir.AluOpType
    maxmsg = max(cv["hc"] + cv["H"] for cv in S["convs"])

    with tile.TileContext(nc) as tc, contextlib.ExitStack() as st:
        cst = st.enter_context(tc.tile_pool(name="cst", bufs=1))
        pstr = st.enter_context(tc.tile_pool(name="pstr", bufs=2,
                                             space="PSUM"))
        psmm = st.enter_context(tc.tile_pool(name="psmm", bufs=1,
                                             space="PSUM"))
        lyr = contextlib.ExitStack()
        wp = lyr.enter_context(tc.tile_pool(name="wp", bufs=8))
        hp = lyr.enter_context(tc.tile_pool(name="hp", bufs=3))
        htp = lyr.enter_context(tc.tile_pool(name="htp", bufs=9))
        hwxp = lyr.enter_context(tc.tile_pool(name="hwxp", bufs=2))
        selp = lyr.enter_context(tc.tile_pool(name="selp", bufs=3))
        lgp = lyr.enter_context(tc.tile_pool(name="lgp", bufs=4))
        hnp = lyr.enter_context(tc.tile_pool(name="hnp", bufs=3))
        gp = lyr.enter_context(tc.tile_pool(name="gp", bufs=2))
        fl = lyr.enter_context(tc.tile_pool(name="fl", bufs=2))

        def cload(t, shape, tag, dtt=dt):
            s = cst.tile(shape, dtt, tag=tag, name=tag)
            nc.sync.dma_start(out=s[:], in_=t.ap())
            return s

        sb_id = cload(t_id, [P, P], "id")
        sb_ar = cload(t_ar, [P, 256], "ar")
        sb_ac = cload(t_ac, [P, 1], "ac")
        sb_eidx = cload(t_eidx, [P, nt_pad * P // 16], "eidx",
                        mybir.dt.int16)
        sb_dstloc = cload(t_dstloc, [P, nt_pad], "dstloc")
        sb_batchloc = cload(t_batchloc, [P, nblk], "batchloc")
        sb_gids = cload(t_gids, [gpc, 1], "gids", mybir.dt.int32)
        sb_bias = {i: cload(t_bias[i], [P, cv["hc"]], f"bias{i}")
                   for i, cv in enumerate(S["convs"])}
        bg_sb = cload(t_bg, [P, S["wg_out"]], "bg")
        bf_sb = [cload(t_bf[j], [P, S["fcs"][j][1]], f"bf{j}")
                 for j in range(5)]
        sb_ald = cst.tile([P, nblk, maxH], dt, tag="ald", name="ald")
        sb_was = {i: cst.tile([P, _ceil(cv["cin"], P), 2 * cv["H"]], dt,
                              tag=f"was{i}", name=f"was{i}")
                  for i, cv in enumerate(S["convs"])}

        # ---- phase A: was_i = W_i @ [A_s | A_d] ----
        with tc.tile_pool(name="wtp", bufs=8) as wtp, \
             tc.tile_pool(name="amp", bufs=8) as amp:
            for i, cv in enumerate(S["convs"]):
                cin, hc, H = cv["cin"], cv["hc"], cv["H"]
                nk, nh = _ceil(cin, P), _ceil(hc, P)
                w_t = []
                for kt in range(nk):
                    kn = min(P, cin - kt * P)
                    wt = wp.tile([P, hc], dt, tag="w")
                    nc.sync.dma_start(out=wt[:kn, :],
                                      in_=t_w[i].ap()[kt*P:kt*P+kn, :])
                    w_t.append((wt, kn))
                wT = []
                for hcc in range(nh):
                    cw = min(P, hc - hcc * P)
                    wTt = wtp.tile([P, maxin], dt, tag="wT")
                    for kt in range(nk):
                        kn = w_t[kt][1]
                        pst = pstr.tile([P, P], dt, tag="tr")
                        nc.tensor.transpose(
                            out=pst[:cw, :kn],
                            in_=w_t[kt][0][:kn, hcc*P:hcc*P+cw],
                            identity=sb_id[:kn, :kn])
                        nc.vector.tensor_copy(out=wTt[:cw, kt*P:kt*P+kn],
                                              in_=pst[:cw, :kn])
                    wT.append((wTt, cw))
                am = []
                for hcc in range(nh):
                    cw = wT[hcc][1]
                    amt = amp.tile([P, 2 * H], dt, tag="am")
                    nc.sync.dma_start(out=amt[:cw, :],
                                      in_=t_amat[i].ap()[hcc*P:hcc*P+cw, :])
                    am.append(amt)
                for ic in range(nk):
                    icw = min(P, cin - ic * P)
                    psw = pstr.tile([P, P], dt, tag="tr")
                    for hcc in range(nh):
                        cw = wT[hcc][1]
                        nc.tensor.matmul(
                            out=psw[:icw, :2*H],
                            lhsT=wT[hcc][0][:cw, ic*P:ic*P+icw],
                            rhs=am[hcc][:cw, :],
                            start=(hcc == 0), stop=(hcc == nh - 1))
                    nc.vector.tensor_copy(out=sb_was[i][:icw, ic, :],
                                          in_=psw[:icw, :2*H])

        # ---- GAT layers (edge psum pools in inner scope) ----
        with tc.tile_pool(name="psacA", bufs=2, space="PSUM") as psacA, \
             tc.tile_pool(name="psacB", bufs=2, space="PSUM") as psacB:
            for i, cv in enumerate(S["convs"]):
                cin, hc, H, C = cv["cin"], cv["hc"], cv["H"], cv["C"]
                tcw = S["tcols"][i]
                nk = _ceil(cin, P)
                mmout = hc + 2 * H
                msgc = hc + H
                assert 512 < msgc <= 1024 and hc >= 512
                bounce, table = t_bounce[tcw], t_table[tcw]

                w_t = []
                for kt in range(nk):
                    kn = min(P, cin - kt * P)
                    wt = wp.tile([P, hc], dt, tag="w")
                    nc.sync.dma_start(out=wt[:kn, :],
                                      in_=t_w[i].ap()[kt*P:kt*P+kn, :])
                    w_t.append((wt, kn))

                # node stage
                for t in range(nblk):
                    h_t = hp.tile([P, cin], dt, tag="h")
                    if i == 0:
                        nc.sync.dma_start(out=h_t[:],
                                          in_=t_x.ap()[t*P:(t+1)*P, :])
                    else:
                        nc.sync.dma_start(
                            out=h_t[:],
                            in_=t_hbuf[(i + 1) % 2].ap()[t*P:(t+1)*P, :cin])
                    hT = []
                    for kt in range(nk):
                        kn = w_t[kt][1]
                        pst = pstr.tile([P, P], dt, tag="tr")
                        nc.tensor.transpose(out=pst[:kn, :],
                                            in_=h_t[:, kt*P:kt*P+kn],
                                            identity=sb_id[:])
                        hTt = htp.tile([P, P], dt, tag="hT")
                        nc.vector.tensor_copy(out=hTt[:kn, :],
                                              in_=pst[:kn, :])
                        hT.append((hTt, kn))
                    pmA = psmm.tile([P, 512], dt, tag="mmA")
                    pmB = psmm.tile([P, 512], dt, tag="mmB")
                    for kt in range(nk):
                        kn = hT[kt][1]
                        stt, spp = (kt == 0), (kt == nk - 1)
                        # bank A: cols 0:512
                        nc.tensor.matmul(out=pmA[:], lhsT=hT[kt][0][:kn, :],
                                         rhs=w_t[kt][0][:kn, :512],
                                         start=stt, stop=spp)
                        # bank B: cols 512:hc then the 2H attention cols;
                        # one group per bank: start on first write, stop on
                        # last.
                        nc.tensor.matmul(out=pmB[:, :hc-512],
                                         lhsT=hT[kt][0][:kn, :],
                                         rhs=w_t[kt][0][:kn, 512:hc],
                                         start=stt, stop=False)
                        nc.tensor.matmul(out=pmB[:, hc-512:mmout-512],
                                         lhsT=hT[kt][0][:kn, :],
                                         rhs=sb_was[i][:kn, kt, :],
                                         start=False, stop=spp)
                    hwx = hwxp.tile([P, tcw], dt, tag="hwx")
                    nc.vector.tensor_copy(out=hwx[:, :512], in_=pmA[:])
                    nc.vector.tensor_copy(out=hwx[:, 512:msgc],
                                          in_=pmB[:, :msgc-512])
                    if tcw > msgc:
                        nc.vector.memset(hwx[:, msgc:tcw], 0)
                    nc.vector.tensor_copy(out=sb_ald[:, t, :H],
                                          in_=pmB[:, msgc-512:mmout-512])
                    nc.sync.dma_start(out=bounce.ap()[t*P:(t+1)*P, :],
                                      in_=hwx[:])

                # exchange
                nc.gpsimd.collective_compute(
                    "AllGather", AT.bypass, replica_groups=groups,
                    ins=[bounce.ap()], outs=[table.ap()])

                # edge stage
                psA = psB = None
                for ch in range(S["nchunks"]):
                    g_t = gp.tile([P, SLOTS, tcw], dt, tag="g")
                    nc.gpsimd.dma_gather(
                        out_ap=g_t[:], in_ap=table.ap(),
                        idxs_ap=sb_eidx[:, ch*(CHUNK//16):(ch+1)*(CHUNK//16)],
                        num_idxs=CHUNK, num_idxs_reg=CHUNK,
                        elem_size=tcw, elem_step=tcw)
                    for s in range(SLOTS):
                        g = ch * SLOTS + s
                        b = min(g // t_blk, nblk - 1)
                        first = (g == b * t_blk)
                        last = (g == (b + 1) * t_blk - 1) if b < nblk - 1 \
                            else (g == nt_pad - 1)
                        dcol = sb_dstloc[:, g:g+1]
                        psrow = pstr.tile([P, P], dt, tag="tr")
                        nc.tensor.transpose(out=psrow[:],
                                            in_=dcol.to_broadcast([P, P]),
                                            identity=sb_id[:])
                        sel2 = selp.tile([P, P], dt, tag="sel2")
                        nc.vector.tensor_tensor(
                            out=sel2[:], in0=sb_ac[:].to_broadcast([P, P]),
                            in1=psrow[:], op=AT.is_equal)
                        selT = selp.tile([P, P], dt, tag="selT")
                        nc.vector.tensor_tensor(
                            out=selT[:], in0=dcol.to_broadcast([P, P]),
                            in1=sb_ar[:, :P], op=AT.is_equal)
                        psal = pstr.tile([P, P], dt, tag="tr")
                        nc.tensor.matmul(out=psal[:, :H], lhsT=sel2[:],
                                         rhs=sb_ald[:, b, :H],
                                         start=True, stop=True)
                        lg = lgp.tile([P, maxH], dt, tag="lg")
                        nc.vector.tensor_tensor(
                            out=lg[:, :H], in0=g_t[:, s, hc:hc+H],
                            in1=psal[:, :H], op=AT.add)
                        lr = lgp.tile([P, maxH], dt, tag="lr")
                        nc.vector.tensor_scalar_mul(out=lr[:, :H],
                                                    in0=lg[:, :H],
                                                    scalar1=0.2)
                        nc.vector.tensor_tensor(out=lr[:, :H], in0=lr[:, :H],
                                                in1=lg[:, :H], op=AT.max)
                        nc.scalar.activation(out=g_t[:, s, hc:hc+H],
                                             in_=lr[:, :H], func=Exp)
                        for h in range(H):
                            wcol = g_t[:, s, hc+h:hc+h+1]
                            dap = g_t[:, s, h*C:(h+1)*C]
                            if h < (H + 1) // 2:
                                nc.vector.tensor_scalar_mul(
                                    out=dap, in0=dap, scalar1=wcol)
                            else:
                                nc.scalar.activation(out=dap, in_=dap,
                                                     func=Copy, scale=wcol)
                        if first:
                            psA = psacA.tile([P, 512], dt, tag="psA")
                            psB = psacB.tile([P, maxmsg - 512], dt,
                                             tag="psB")
                        nc.tensor.matmul(out=psA[:], lhsT=selT[:],
                                         rhs=g_t[:, s, :512],
                                         start=first, stop=last)
                        nc.tensor.matmul(out=psB[:, :msgc-512], lhsT=selT[:],
                                         rhs=g_t[:, s, 512:msgc],
                                         start=first, stop=last)
                        if last:
                            def pslice(c0, c1):
                                assert (c0 >= 512) == (c1 > 512)
                                if c1 <= 512:
                                    return psA[:, c0:c1]
                                return psB[:, c0-512:c1-512]
                            sden = fl.tile([P, maxH], dt, tag="sden")
                            nc.vector.tensor_scalar_add(
                                out=sden[:, :H], in0=pslice(hc, hc + H),
                                scalar1=1e-16)
                            nc.vector.reciprocal(out=sden[:, :H],
                                                 in_=sden[:, :H])
                            hn = hnp.tile([P, hc], dt, tag="hn")
                            for h in range(H):
                                c0, c1 = h * C, (h + 1) * C
                                cuts = [c0, c1] if (c0 >= 512 or c1 <= 512) \
                                    else [c0, 512, c1]
                                for a0, a1 in zip(cuts[:-1], cuts[1:]):
                                    nc.vector.tensor_scalar(
                                        out=hn[:, a0:a1],
                                        in0=pslice(a0, a1),
                                        scalar1=sden[:, h:h+1],
                                        scalar2=None, op0=AT.mult)
                            nc.vector.tensor_tensor(
                                out=hn[:], in0=hn[:],
                                in1=sb_bias[i][:, :hc], op=AT.add)
                            nc.vector.tensor_scalar_max(
                                out=hn[:], in0=hn[:], scalar1=0.0)
                            nc.sync.dma_start(
                                out=t_hbuf[i % 2].ap()[b*P:(b+1)*P, :hc],
                                in_=hn[:])

            # ---- pooling ----
            for half in range(S["ghalves"]):
                ppA = psmm.tile([P, 512], dt, tag="mmA")
                ppB = psmm.tile([P, 512], dt, tag="mmB")
                for t in range(nblk):
                    h_t = hp.tile([P, hc4], dt, tag="h")
                    nc.sync.dma_start(
                        out=h_t[:], in_=t_hbuf[1].ap()[t*P:(t+1)*P, :hc4])
                    selG = selp.tile([P, P], dt, tag="selG")
                    nc.vector.tensor_tensor(
                        out=selG[:],
                        in0=sb_batchloc[:, t:t+1].to_broadcast([P, P]),
                        in1=sb_ar[:, half*P:(half+1)*P], op=AT.is_equal)
                    nc.tensor.matmul(out=ppA[:], lhsT=selG[:],
                                     rhs=h_t[:, :512],
                                     start=(t == 0), stop=(t == nblk - 1))
                    nc.tensor.matmul(out=ppB[:, :hc4-512], lhsT=selG[:],
                                     rhs=h_t[:, 512:hc4],
                                     start=(t == 0), stop=(t == nblk - 1))
                pl = fl.tile([P, hc4], dt, tag="pl")
                nc.vector.tensor_copy(out=pl[:, :512], in_=ppA[:])
                nc.vector.tensor_copy(out=pl[:, 512:hc4],
                                      in_=ppB[:, :hc4-512])
                rows = min(P, B - half * P)
                nc.sync.dma_start(out=t_pin.ap()[half*P:half*P+rows, :],
                                  in_=pl[:rows, :])
            nc.gpsimd.collective_compute(
                "AllReduce", AT.add, replica_groups=groups,
                ins=[t_pin.ap()], outs=[t_pout.ap()])
        lyr.close()

        # ---- FC head (layer pools closed) ----
        with tc.tile_pool(name="fcz", bufs=20) as fcz, \
             tc.tile_pool(name="fcw", bufs=3) as fcw, \
             tc.tile_pool(name="fcs", bufs=2) as fcs, \
             tc.tile_pool(name="fcps", bufs=1, space="PSUM") as fcps:
            g = gpc
            myp = fcs.tile([g, hc4], dt, tag="myp")
            nc.gpsimd.indirect_dma_start(
                out=myp[:], out_offset=None, in_=t_pout.ap(),
                in_offset=bass.IndirectOffsetOnAxis(ap=sb_gids[:, :1],
                                                    axis=0))
            dsc = fcs.tile([g, S["desc_dim"]], dt, tag="dsc")
            nc.sync.dma_start(out=dsc[:], in_=t_desc.ap())

            def transpose_rows(z_ap, n_cols):
                out = []
                for kt in range(_ceil(n_cols, P)):
                    kn = min(P, n_cols - kt * P)
                    pst = pstr.tile([P, P], dt, tag="tr")
                    nc.tensor.transpose(out=pst[:kn, :g],
                                        in_=z_ap[:, kt*P:kt*P+kn],
                                        identity=sb_id[:g, :g])
                    zt = fcz.tile([P, g], dt, tag="zt")
                    nc.vector.tensor_copy(out=zt[:kn, :], in_=pst[:kn, :g])
                    out.append((zt, kn))
                return out

            def fc_matmul(zt_list, w_tensor, n_out, bias_sb, relu):
                ps = fcps.tile([g, n_out], dt, tag="fps")
                r = 0
                for kt, (zt, kn) in enumerate(zt_list):
                    wt = fcw.tile([P, n_out], dt, tag="fw")
                    nc.sync.dma_start(out=wt[:kn, :],
                                      in_=w_tensor.ap()[r:r+kn, :])
                    r += kn
                    for (c0, c1) in _col_chunks(n_out):
                        nc.tensor.matmul(out=ps[:, c0:c1],
                                         lhsT=zt[:kn, :g],
                                         rhs=wt[:kn, c0:c1],
                                         start=(kt == 0),
                                         stop=(kt == len(zt_list) - 1))
                z = fcs.tile([g, n_out], dt, tag="z")
                nc.vector.tensor_tensor(out=z[:], in0=ps[:],
                                        in1=bias_sb[:g, :n_out], op=AT.add)
                if relu:
                    nc.vector.tensor_scalar_max(out=z[:], in0=z[:],
                                                scalar1=0.0)
                return z

            zt = transpose_rows(myp[:], hc4)
            zg = fc_matmul(zt, t_wg, S["wg_out"], bg_sb, relu=False)
            zt1 = transpose_rows(zg[:], S["wg_out"]) \
                + transpose_rows(dsc[:], S["desc_dim"])
            z = fc_matmul(zt1, t_wf[0], S["fcs"][0][1], bf_sb[0], relu=True)
            for j in range(1, 5):
                ztj = transpose_rows(z[:], S["fcs"][j][0])
                z = fc_matmul(ztj, t_wf[j], S["fcs"][j][1], bf_sb[j],
                              relu=(j < 4))
            nc.gpsimd.indirect_dma_start(
                out=t_out.ap(),
                out_offset=bass.IndirectOffsetOnAxis(ap=sb_gids[:, :1],
                                                     axis=0),
                in_=z[:], in_offset=None)

    nc.compile()
    return nc


# ---------------------------------------------------------------------------
# Entry point
# ---------------------------------------------------------------------------

_CACHE = {}


def kernel(**inputs):
    from concourse import bass_utils

    S, in_maps = _prep(inputs)
    key = repr(sorted(S.items()))
    if key not in _CACHE:
        _CACHE[key] = _build(S)
    nc = _CACHE[key]
    res = bass_utils.run_bass_kernel_spmd(nc, in_maps,
                                          core_ids=list(range(NCORES)))
    B, gpc = S["B"], S["gpc"]
    out = np.zeros((B, 1), np.float32)
    for c in range(NCORES):
        out[c*gpc:(c+1)*gpc] = res.results[c]["out"][c*gpc:(c+1)*gpc]
    return out


# revision 14
# speedup vs baseline: 1.2201x; 1.2201x over previous
"""Trainium2 Bass kernel for nn_GAT_Model (4x GATConv + global_add_pool + MLP).

Strategy (8 NeuronCores, SPMD single program):
  - Nodes partitioned into 8 equal ranges (1250/core, padded to 1280 local
    slots = 10 dst-blocks of 128).  Edges (incl. self-loops) are assigned to
    the core owning their dst node, sorted by dst, grouped into dst-blocks
    with a uniform tile count T_blk per block (host-padded; pad edges carry
    dstloc=-1 so their selector rows are all-zero and they contribute
    nothing).
  - Per layer: each core matmuls its own node rows producing
    [h@W | al_src | al_dst] (attention coefficients folded into the matmul
    via device-computed W@[A_s|A_d]), AllGathers the [hW | al_src] table,
    then processes its edges: dma_gather of source rows, per-edge softmax
    weights exp(leakyrelu(al_s[src]+al_d[dst])) (logits are O(1) here so no
    max subtraction is needed), aggregation of numerator and denominator via
    one-hot selector matmuls accumulating in PSUM.
  - Pooling via 256-graph selector matmuls -> partial sums -> AllReduce.
  - MLP head sharded by graph (32 graphs/core), final scatter by row ids.

kernel(**inputs) takes FULL inputs and returns the FULL [256,1] output.
"""

import math

import numpy as np

P = 128
NCORES = 8
CHUNK = 1024            # edges per dma_gather call
SLOTS = CHUNK // P      # gather slots (tiles of 128 edges) per chunk


# ---------------------------------------------------------------------------
# Host-side preprocessing (integer index manipulation only)
# ---------------------------------------------------------------------------

def _prep(inputs):
    x = np.asarray(inputs["x"], np.float32)
    edge_index = np.asarray(inputs["edge_index"], np.int64)
    batch = np.asarray(inputs["batch"], np.int64)
    desc = np.asarray(inputs["desc"], np.float32)

    N = x.shape[0]
    B = desc.shape[0]
    npc = N // NCORES
    assert npc * NCORES == N
    nblk = (npc + P - 1) // P
    locpad = nblk * P
    trows = NCORES * locpad
    gpc = B // NCORES
    assert gpc * NCORES == B

    convs = []
    for i in range(1, 5):
        W = np.asarray(inputs[f"W{i}"], np.float32)
        a_s = np.asarray(inputs[f"as{i}"], np.float32)
        a_d = np.asarray(inputs[f"ad{i}"], np.float32)
        b = np.asarray(inputs[f"b{i}"], np.float32)
        H, C = a_s.shape
        convs.append(dict(W=W, a_s=a_s, a_d=a_d, b=b, H=H, C=C,
                          cin=W.shape[0], hc=H * C))

    loops = np.arange(N, dtype=np.int64)
    src = np.concatenate([np.asarray(edge_index[0]), loops])
    dst = np.concatenate([np.asarray(edge_index[1]), loops])

    core_of = dst // npc
    src_tr = locpad * (src // npc) + (src % npc)   # table row per src node
    order = np.argsort(dst, kind="stable")

    blk_edges = [[[] for _ in range(nblk)] for _ in range(NCORES)]
    for e in order:
        c = int(core_of[e])
        d = int(dst[e]) - c * npc
        blk_edges[c][d // P].append((int(src_tr[e]), d - (d // P) * P))

    t_blk = max((len(bl) + P - 1) // P for cb in blk_edges for bl in cb)
    nt = nblk * t_blk
    nt_pad = ((nt + SLOTS - 1) // SLOTS) * SLOTS
    nchunks = nt_pad // SLOTS
    zrow = locpad - 1                      # pad rows are inert (dstloc=-1)

    eidx = np.full((NCORES, nt_pad * P), zrow, np.int16)
    dstloc = np.full((NCORES, nt_pad, P), -1.0, np.float32)
    for c in range(NCORES):
        for b in range(nblk):
            for j, (s_tr, d_loc) in enumerate(blk_edges[c][b]):
                g = b * t_blk + j // P
                eidx[c, g * P + (j % P)] = s_tr
                dstloc[c, g, j % P] = d_loc
    eidx_w = np.zeros((NCORES, P, nt_pad * P // 16), np.int16)
    for c in range(NCORES):
        w = eidx[c].reshape(-1, 16).T
        eidx_w[c] = np.tile(w, (8, 1))
    dstloc_t = np.ascontiguousarray(dstloc.transpose(0, 2, 1))
    dstrows = np.ascontiguousarray(dstloc)            # [NCORES, nt_pad, P]

    batchloc = np.full((NCORES, P, nblk), -1.0, np.float32)
    for c in range(NCORES):
        bl = batch[c * npc:(c + 1) * npc].astype(np.float32)
        for t in range(nblk):
            seg = bl[t * P:(t + 1) * P]
            batchloc[c, :len(seg), t] = seg

    xs = np.zeros((NCORES, locpad, x.shape[1]), np.float32)
    for c in range(NCORES):
        xs[c, :npc] = x[c * npc:(c + 1) * npc]

    amats = []
    for cv in convs:
        H, C, hc = cv["H"], cv["C"], cv["hc"]
        A = np.zeros((hc, 2 * H), np.float32)
        for h in range(H):
            A[h * C:(h + 1) * C, h] = cv["a_s"][h]
            A[h * C:(h + 1) * C, H + h] = cv["a_d"][h]
        amats.append(A)

    S = dict(
        N=N, B=B, npc=npc, nblk=nblk, locpad=locpad, trows=trows,
        gpc=gpc, ghalves=(B + P - 1) // P, t_blk=t_blk, nt=nt,
        nt_pad=nt_pad, nchunks=nchunks, desc_dim=desc.shape[1],
        xin=x.shape[1],
        convs=tuple(dict(H=c["H"], C=c["C"], cin=c["cin"], hc=c["hc"])
                    for c in convs),
        fcs=tuple((np.asarray(inputs[f"Wf{j}"]).shape[0],
                   np.asarray(inputs[f"Wf{j}"]).shape[1])
                  for j in range(1, 6)),
        wg_in=np.asarray(inputs["Wg"]).shape[0],
        wg_out=np.asarray(inputs["Wg"]).shape[1],
    )
    S["tcols"] = tuple(int(math.ceil((c["hc"] + c["H"]) * 4 / 256) * 64)
                       for c in S["convs"])

    arangerow = np.tile(np.arange(256, dtype=np.float32), (P, 1))
    arangecol = np.arange(P, dtype=np.float32).reshape(P, 1)
    ident = np.eye(P, dtype=np.float32)

    in_maps = []
    for c in range(NCORES):
        m = dict(
            x=xs[c],
            eidx=eidx_w[c],
            dstloc=dstloc_t[c],
            dstrows=dstrows[c],
            batchloc=batchloc[c],
            gids=(c * gpc + np.arange(gpc, dtype=np.int32)).reshape(gpc, 1),
            desc=np.ascontiguousarray(desc[c * gpc:(c + 1) * gpc]),
            arangerow=arangerow, arangecol=arangecol, ident=ident,
            wg=np.asarray(inputs["Wg"], np.float32),
            bg=np.tile(np.asarray(inputs["bg"], np.float32), (P, 1)),
        )
        for i, cv in enumerate(convs):
            m[f"w{i+1}"] = cv["W"]
            m[f"amat{i+1}"] = amats[i]
            m[f"bias{i+1}"] = np.tile(cv["b"], (P, 1))
        for j in range(1, 6):
            m[f"wf{j}"] = np.asarray(inputs[f"Wf{j}"], np.float32)
            m[f"bf{j}"] = np.tile(np.asarray(inputs[f"bf{j}"], np.float32),
                                  (P, 1))
        in_maps.append(m)
    return S, in_maps


# ---------------------------------------------------------------------------
# Device program
# ---------------------------------------------------------------------------

def _ceil(a, b):
    return (a + b - 1) // b


def _col_chunks(n, lim=512):
    out, c = [], 0
    while c < n:
        out.append((c, min(c + lim, n)))
        c = min(c + lim, n)
    return out


def _build(S, enable_asserts=False, single=False):
    import contextlib

    import concourse.bass as bass
    import concourse.mybir as mybir
    import concourse.tile as tile
    from concourse import bacc

    dt = mybir.dt.float32
    nblk, t_blk, nt_pad = S["nblk"], S["t_blk"], S["nt_pad"]
    locpad, trows = S["locpad"], S["trows"]
    gpc, B = S["gpc"], S["B"]
    maxH = max(cv["H"] for cv in S["convs"])

    ndev = 1 if single else NCORES
    nc = bacc.Bacc("TRN2", target_bir_lowering=False, debug=False,
                   enable_asserts=enable_asserts, num_devices=ndev)

    # ---- I/O ----
    t_x = nc.dram_tensor("x", [locpad, S["xin"]], dt, kind="ExternalInput")
    t_eidx = nc.dram_tensor("eidx", [P, nt_pad * P // 16], mybir.dt.int16,
                            kind="ExternalInput")
    t_dstloc = nc.dram_tensor("dstloc", [P, nt_pad], dt, kind="ExternalInput")
    t_dstrows = nc.dram_tensor("dstrows", [nt_pad, P], dt,
                               kind="ExternalInput")
    t_batchloc = nc.dram_tensor("batchloc", [P, nblk], dt,
                                kind="ExternalInput")
    t_gids = nc.dram_tensor("gids", [gpc, 1], mybir.dt.int32,
                            kind="ExternalInput")
    t_desc = nc.dram_tensor("desc", [gpc, S["desc_dim"]], dt,
                            kind="ExternalInput")
    t_ar = nc.dram_tensor("arangerow", [P, 256], dt, kind="ExternalInput")
    t_ac = nc.dram_tensor("arangecol", [P, 1], dt, kind="ExternalInput")
    t_id = nc.dram_tensor("ident", [P, P], dt, kind="ExternalInput")
    t_wg = nc.dram_tensor("wg", [S["wg_in"], S["wg_out"]], dt,
                          kind="ExternalInput")
    t_bg = nc.dram_tensor("bg", [P, S["wg_out"]], dt, kind="ExternalInput")
    t_w, t_amat, t_bias, t_wf, t_bf = {}, {}, {}, {}, {}
    for i, cv in enumerate(S["convs"]):
        t_w[i] = nc.dram_tensor(f"w{i+1}", [cv["cin"], cv["hc"]], dt,
                                kind="ExternalInput")
        t_amat[i] = nc.dram_tensor(f"amat{i+1}", [cv["hc"], 2 * cv["H"]], dt,
                                   kind="ExternalInput")
        t_bias[i] = nc.dram_tensor(f"bias{i+1}", [P, cv["hc"]], dt,
                                   kind="ExternalInput")
    for j, (fi, fo) in enumerate(S["fcs"]):
        t_wf[j] = nc.dram_tensor(f"wf{j+1}", [fi, fo], dt,
                                 kind="ExternalInput")
        t_bf[j] = nc.dram_tensor(f"bf{j+1}", [P, fo], dt,
                                 kind="ExternalInput")
    t_out = nc.dram_tensor("out", [B, 1], dt, kind="ExternalOutput")

    # ---- internal DRAM ----
    tc_widths = sorted(set(S["tcols"]))
    t_bounce = {w: nc.dram_tensor(f"bounce{w}", [locpad, w], dt,
                                  kind="Internal") for w in tc_widths}
    t_table = {w: nc.dram_tensor(f"table{w}", [trows, w], dt,
                                 kind="Internal", addr_space="Shared")
               for w in tc_widths}
    maxin = max(cv["cin"] for cv in S["convs"])
    t_hbuf = [nc.dram_tensor(f"hbuf{k}", [locpad, maxin], dt,
                             kind="Internal") for k in range(2)]
    hc4 = S["convs"][3]["hc"]
    t_pin = nc.dram_tensor("pooled_in", [B, hc4], dt, kind="Internal")
    t_pout = nc.dram_tensor("pooled_out", [B, hc4], dt, kind="Internal",
                            addr_space="Shared")

    groups = [list(range(NCORES))]
    Exp = mybir.ActivationFunctionType.Exp
    Copy = mybir.ActivationFunctionType.Copy
    AT = mybir.AluOpType
    maxmsg = max(cv["hc"] + cv["H"] for cv in S["convs"])

    with tile.TileContext(nc) as tc, contextlib.ExitStack() as st:
        cst = st.enter_context(tc.tile_pool(name="cst", bufs=1))
        pstr = st.enter_context(tc.tile_pool(name="pstr", bufs=3,
                                             space="PSUM"))
        lyr = contextlib.ExitStack()
        wp = lyr.enter_context(tc.tile_pool(name="wp", bufs=8))
        hp = lyr.enter_context(tc.tile_pool(name="hp", bufs=3))
        htp = lyr.enter_context(tc.tile_pool(name="htp", bufs=2))
        hwxp = lyr.enter_context(tc.tile_pool(name="hwxp", bufs=2))
        selp = lyr.enter_context(tc.tile_pool(name="selp", bufs=2))
        lgp = lyr.enter_context(tc.tile_pool(name="lgp", bufs=4))
        hnp = lyr.enter_context(tc.tile_pool(name="hnp", bufs=2))
        gp = lyr.enter_context(tc.tile_pool(name="gp", bufs=2))
        fl = lyr.enter_context(tc.tile_pool(name="fl", bufs=2))

        def cload(t, shape, tag, dtt=dt, pool=None):
            s = (pool or cst).tile(shape, dtt, tag=tag, name=tag)
            nc.sync.dma_start(out=s[:], in_=t.ap())
            return s

        sb_id = cload(t_id, [P, P], "id")
        sb_ar = cload(t_ar, [P, 256], "ar")
        sb_ac = cload(t_ac, [P, 1], "ac")
        sb_eidx = cload(t_eidx, [P, nt_pad * P // 16], "eidx",
                        mybir.dt.int16)
        sb_dstloc = cload(t_dstloc, [P, nt_pad], "dstloc")
        sb_batchloc = cload(t_batchloc, [P, nblk], "batchloc")
        sb_gids = cload(t_gids, [gpc, 1], "gids", mybir.dt.int32)
        sb_bias = {i: cload(t_bias[i], [P, cv["hc"]], f"bias{i}")
                   for i, cv in enumerate(S["convs"])}
        sb_ald = cst.tile([P, nblk, maxH], dt, tag="ald", name="ald")
        sb_was = {i: cst.tile([P, _ceil(cv["cin"], P), 2 * cv["H"]], dt,
                              tag=f"was{i}", name=f"was{i}")
                  for i, cv in enumerate(S["convs"])}
        for i in sb_was:
            nc.vector.memset(sb_was[i][:], 0)

        # ---- phase A: was_i = W_i @ [A_s | A_d] ----
        with tc.tile_pool(name="wtp", bufs=7) as wtp, \
             tc.tile_pool(name="amp", bufs=8) as amp:
            for i, cv in enumerate(S["convs"]):
                cin, hc, H = cv["cin"], cv["hc"], cv["H"]
                nk, nh = _ceil(cin, P), _ceil(hc, P)
                w_t = []
                for kt in range(nk):
                    kn = min(P, cin - kt * P)
                    wt = wp.tile([P, hc], dt, tag="w")
                    nc.sync.dma_start(out=wt[:kn, :],
                                      in_=t_w[i].ap()[kt*P:kt*P+kn, :])
                    w_t.append((wt, kn))
                wT = []
                for hcc in range(nh):
                    cw = min(P, hc - hcc * P)
                    wTt = wtp.tile([P, maxin], dt, tag="wT")
                    for kt in range(nk):
                        kn = w_t[kt][1]
                        pst = pstr.tile([P, 512], dt, tag="tr")
                        nc.tensor.transpose(
                            out=pst[:cw, :kn],
                            in_=w_t[kt][0][:kn, hcc*P:hcc*P+cw],
                            identity=sb_id[:kn, :kn])
                        nc.vector.tensor_copy(out=wTt[:cw, kt*P:kt*P+kn],
                                              in_=pst[:cw, :kn])
                    wT.append((wTt, cw))
                am = []
                for hcc in range(nh):
                    cw = wT[hcc][1]
                    amt = amp.tile([P, 2 * H], dt, tag="am")
                    nc.sync.dma_start(out=amt[:cw, :],
                                      in_=t_amat[i].ap()[hcc*P:hcc*P+cw, :])
                    am.append(amt)
                for ic in range(nk):
                    icw = min(P, cin - ic * P)
                    psw = pstr.tile([P, 512], dt, tag="tr")
                    for hcc in range(nh):
                        cw = wT[hcc][1]
                        nc.tensor.matmul(
                            out=psw[:icw, :2*H],
                            lhsT=wT[hcc][0][:cw, ic*P:ic*P+icw],
                            rhs=am[hcc][:cw, :],
                            start=(hcc == 0), stop=(hcc == nh - 1))
                    nc.vector.tensor_copy(out=sb_was[i][:icw, ic, :],
                                          in_=psw[:icw, :2*H])

        # ---- GAT layers (edge psum pools in inner scope) ----
        with tc.tile_pool(name="psacA", bufs=2, space="PSUM") as psacA, \
             tc.tile_pool(name="psacB", bufs=2, space="PSUM") as psacB:
            for i, cv in enumerate(S["convs"]):
                cin, hc, H, C = cv["cin"], cv["hc"], cv["H"], cv["C"]
                tcw = S["tcols"][i]
                nk = _ceil(cin, P)
                mmout = hc + 2 * H
                msgc = hc + H
                assert 512 < msgc <= 1024 and hc >= 512
                bounce, table = t_bounce[tcw], t_table[tcw]

                w_t = []
                for kt in range(nk):
                    kn = min(P, cin - kt * P)
                    wt = wp.tile([P, hc], dt, tag="w")
                    if kn < P:
                        z0 = (kn // 32) * 32
                        for zp in range(z0, P, 32):
                            nc.vector.memset(wt[zp:zp+32, :], 0)
                    nc.sync.dma_start(out=wt[:kn, :],
                                      in_=t_w[i].ap()[kt*P:kt*P+kn, :])
                    w_t.append((wt, kn))

                # node stage
                for t in range(nblk):
                    h_t = hp.tile([P, nk * P], dt, tag="h")
                    if cin < nk * P:
                        nc.vector.memset(h_t[:, cin:], 0)
                    if i == 0:
                        nc.sync.dma_start(out=h_t[:, :cin],
                                          in_=t_x.ap()[t*P:(t+1)*P, :])
                    else:
                        nc.sync.dma_start(
                            out=h_t[:, :cin],
                            in_=t_hbuf[(i + 1) % 2].ap()[t*P:(t+1)*P, :cin])
                    hT8 = htp.tile([P, nk * P], dt, tag="hT")
                    for grp in range(_ceil(nk, 4)):
                        gsec = min(4, nk - grp * 4)
                        pst = pstr.tile([P, 512], dt, tag="tr")
                        for j in range(gsec):
                            kt = grp * 4 + j
                            nc.tensor.transpose(out=pst[:, j*P:(j+1)*P],
                                                in_=h_t[:, kt*P:(kt+1)*P],
                                                identity=sb_id[:])
                        nc.vector.tensor_copy(
                            out=hT8[:, grp*512:grp*512+gsec*P],
                            in_=pst[:, :gsec*P])
                    pmA = psacA.tile([P, 512], dt, tag="psA")
                    pmB = psacB.tile([P, 384], dt, tag="psB")
                    for kt in range(nk):
                        stt, spp = (kt == 0), (kt == nk - 1)
                        lhs = hT8[:, kt*P:(kt+1)*P]
                        nc.tensor.matmul(out=pmA[:], lhsT=lhs,
                                         rhs=w_t[kt][0][:, :512],
                                         start=stt, stop=spp)
                        nc.tensor.matmul(out=pmB[:, :hc-512], lhsT=lhs,
                                         rhs=w_t[kt][0][:, 512:hc],
                                         start=stt, stop=False)
                        nc.tensor.matmul(out=pmB[:, hc-512:mmout-512],
                                         lhsT=lhs,
                                         rhs=sb_was[i][:, kt, :],
                                         start=False, stop=spp)
                    hwx = hwxp.tile([P, tcw], dt, tag="hwx")
                    nc.vector.tensor_copy(out=hwx[:, :512], in_=pmA[:])
                    nc.vector.tensor_copy(out=hwx[:, 512:msgc],
                                          in_=pmB[:, :msgc-512])
                    if tcw > msgc:
                        nc.vector.memset(hwx[:, msgc:tcw], 0)
                    nc.vector.tensor_copy(out=sb_ald[:, t, :H],
                                          in_=pmB[:, msgc-512:mmout-512])
                    nc.sync.dma_start(out=bounce.ap()[t*P:(t+1)*P, :],
                                      in_=hwx[:])

                # exchange
                if single:
                    for t in range(nblk):
                        ccm = hwxp.tile([P, tcw], dt, tag="hwx", name="ccm")
                        nc.sync.dma_start(out=ccm[:],
                                          in_=bounce.ap()[t*P:(t+1)*P, :])
                        nc.sync.dma_start(out=table.ap()[t*P:(t+1)*P, :],
                                          in_=ccm[:])
                else:
                    nc.gpsimd.collective_compute(
                        "AllGather", AT.bypass, replica_groups=groups,
                        ins=[bounce.ap()], outs=[table.ap()])

                # edge stage
                psA = psB = None
                for ch in range(S["nchunks"]):
                    g0 = ch * SLOTS
                    g_t = gp.tile([P, SLOTS, tcw], dt, tag="g")
                    nc.gpsimd.dma_gather(
                        out_ap=g_t[:], in_ap=table.ap(),
                        idxs_ap=sb_eidx[:, ch*(CHUNK//16):(ch+1)*(CHUNK//16)],
                        num_idxs=CHUNK, num_idxs_reg=CHUNK,
                        elem_size=tcw, elem_step=tcw)
                    # batched selT: sel8[p, s, d] = (dstloc[p, g0+s] == d)
                    sel8 = selp.tile([P, SLOTS, P], dt, tag="sel8")
                    nc.vector.tensor_tensor(
                        out=sel8[:],
                        in0=sb_dstloc[:, g0:g0+SLOTS].unsqueeze(2)
                        .to_broadcast([P, SLOTS, P]),
                        in1=sb_ar[:, :P].unsqueeze(1)
                        .to_broadcast([P, SLOTS, P]),
                        op=AT.is_equal)
                    # batched sel2: dst rows via broadcast DMA, then one
                    # all-SBUF is_equal (2x-eligible)
                    dstr8 = selp.tile([P, SLOTS, P], dt, tag="dstr8")
                    nc.sync.dma_start(
                        out=dstr8[:],
                        in_=t_dstrows.ap()[g0:g0+SLOTS, :]
                        .partition_broadcast(P))
                    sel28 = selp.tile([P, SLOTS, P], dt, tag="sel28")
                    nc.vector.tensor_tensor(
                        out=sel28[:],
                        in0=sb_ac[:].unsqueeze(1)
                        .to_broadcast([P, SLOTS, P]),
                        in1=dstr8[:], op=AT.is_equal)
                    # batched alde
                    psal8 = pstr.tile([P, 512], dt, tag="tr")
                    for sg in range(SLOTS):
                        b = min((g0 + sg) // t_blk, nblk - 1)
                        nc.tensor.matmul(out=psal8[:, sg*H:(sg+1)*H],
                                         lhsT=sel28[:, sg, :],
                                         rhs=sb_ald[:, b, :H],
                                         start=(sg == 0),
                                         stop=(sg == SLOTS - 1))
                    # batched logits -> exp weights (into gather tile)
                    lg8 = lgp.tile([P, SLOTS, maxH], dt, tag="lg")
                    nc.vector.tensor_tensor(
                        out=lg8[:, :, :H], in0=g_t[:, :, hc:hc+H],
                        in1=psal8[:, :SLOTS*H]
                        .rearrange("p (s h) -> p s h", s=SLOTS),
                        op=AT.add)
                    lr8 = lgp.tile([P, SLOTS, maxH], dt, tag="lr")
                    nc.vector.tensor_scalar_mul(out=lr8[:, :, :H],
                                                in0=lg8[:, :, :H],
                                                scalar1=0.2)
                    nc.vector.tensor_tensor(out=lr8[:, :, :H],
                                            in0=lr8[:, :, :H],
                                            in1=lg8[:, :, :H], op=AT.max)
                    nc.scalar.activation(out=g_t[:, :, hc:hc+H],
                                         in_=lr8[:, :, :H], func=Exp)
                    # msg scaling: DVE takes slots [0:5) batched, ACT the
                    # rest per (slot, head) with per-partition scale
                    DV = 5
                    nc.vector.tensor_tensor(
                        out=g_t[:, 0:DV, 0:hc]
                        .rearrange("p s (h c) -> p s h c", h=H),
                        in0=g_t[:, 0:DV, 0:hc]
                        .rearrange("p s (h c) -> p s h c", h=H),
                        in1=g_t[:, 0:DV, hc:hc+H].unsqueeze(3)
                        .to_broadcast([P, DV, H, C]),
                        op=AT.mult)
                    for sg in range(DV, SLOTS):
                        for h in range(H):
                            nc.scalar.activation(
                                out=g_t[:, sg, h*C:(h+1)*C],
                                in_=g_t[:, sg, h*C:(h+1)*C], func=Copy,
                                scale=g_t[:, sg, hc+h:hc+h+1])
                    for s in range(SLOTS):
                        g = ch * SLOTS + s
                        b = min(g // t_blk, nblk - 1)
                        first = (g == b * t_blk)
                        last = (g == (b + 1) * t_blk - 1) if b < nblk - 1 \
                            else (g == nt_pad - 1)
                        if first:
                            psA = psacA.tile([P, 512], dt, tag="psA")
                            psB = psacB.tile([P, 384], dt, tag="psB")
                        nc.tensor.matmul(out=psA[:], lhsT=sel8[:, s, :],
                                         rhs=g_t[:, s, :512],
                                         start=first, stop=last)
                        nc.tensor.matmul(out=psB[:, :msgc-512],
                                         lhsT=sel8[:, s, :],
                                         rhs=g_t[:, s, 512:msgc],
                                         start=first, stop=last)
                        if last:
                            def pslice(c0, c1):
                                assert (c0 >= 512) == (c1 > 512)
                                if c1 <= 512:
                                    return psA[:, c0:c1]
                                return psB[:, c0-512:c1-512]
                            sden = fl.tile([P, maxH], dt, tag="sden")
                            nc.vector.tensor_scalar_add(
                                out=sden[:, :H], in0=pslice(hc, hc + H),
                                scalar1=1e-16)
                            nc.vector.reciprocal(out=sden[:, :H],
                                                 in_=sden[:, :H])
                            hn = hnp.tile([P, hc], dt, tag="hn")
                            for h in range(H):
                                c0, c1 = h * C, (h + 1) * C
                                cuts = [c0, c1] if (c0 >= 512 or c1 <= 512) \
                                    else [c0, 512, c1]
                                for a0, a1 in zip(cuts[:-1], cuts[1:]):
                                    nc.vector.tensor_scalar(
                                        out=hn[:, a0:a1],
                                        in0=pslice(a0, a1),
                                        scalar1=sden[:, h:h+1],
                                        scalar2=None, op0=AT.mult)
                            nc.vector.tensor_tensor(
                                out=hn[:], in0=hn[:],
                                in1=sb_bias[i][:, :hc], op=AT.add)
                            nc.vector.tensor_scalar_max(
                                out=hn[:], in0=hn[:], scalar1=0.0)
                            nc.sync.dma_start(
                                out=t_hbuf[i % 2].ap()[b*P:(b+1)*P, :hc],
                                in_=hn[:])

            # ---- pooling ----
            for half in range(S["ghalves"]):
                ppA = psacA.tile([P, 512], dt, tag="psA")
                ppB = psacB.tile([P, 384], dt, tag="psB")
                for t in range(nblk):
                    h_t = hp.tile([P, hc4], dt, tag="h")
                    nc.sync.dma_start(
                        out=h_t[:], in_=t_hbuf[1].ap()[t*P:(t+1)*P, :hc4])
                    selG = selp.tile([P, P], dt, tag="selG")
                    nc.vector.tensor_tensor(
                        out=selG[:],
                        in0=sb_batchloc[:, t:t+1].to_broadcast([P, P]),
                        in1=sb_ar[:, half*P:(half+1)*P], op=AT.is_equal)
                    nc.tensor.matmul(out=ppA[:], lhsT=selG[:],
                                     rhs=h_t[:, :512],
                                     start=(t == 0), stop=(t == nblk - 1))
                    nc.tensor.matmul(out=ppB[:, :hc4-512], lhsT=selG[:],
                                     rhs=h_t[:, 512:hc4],
                                     start=(t == 0), stop=(t == nblk - 1))
                pl = fl.tile([P, hc4], dt, tag="pl")
                nc.vector.tensor_copy(out=pl[:, :512], in_=ppA[:])
                nc.vector.tensor_copy(out=pl[:, 512:hc4],
                                      in_=ppB[:, :hc4-512])
                rows = min(P, B - half * P)
                nc.sync.dma_start(out=t_pin.ap()[half*P:half*P+rows, :],
                                  in_=pl[:rows, :])
            if single:
                for half in range(S["ghalves"]):
                    ccm2 = fl.tile([P, hc4], dt, tag="pl", name="ccm2")
                    nc.sync.dma_start(out=ccm2[:],
                                      in_=t_pin.ap()[half*P:(half+1)*P, :])
                    nc.sync.dma_start(out=t_pout.ap()[half*P:(half+1)*P, :],
                                      in_=ccm2[:])
            else:
                nc.gpsimd.collective_compute(
                    "AllReduce", AT.add, replica_groups=groups,
                    ins=[t_pin.ap()], outs=[t_pout.ap()])
        lyr.close()

        # ---- FC head (layer pools closed) ----
        with tc.tile_pool(name="fcz", bufs=20) as fcz, \
             tc.tile_pool(name="fcw", bufs=3) as fcw, \
             tc.tile_pool(name="fcs", bufs=2) as fcs, \
             tc.tile_pool(name="fcps", bufs=1, space="PSUM") as fcps:
            g = gpc
            bg_sb = cload(t_bg, [P, S["wg_out"]], "bg", pool=fcs)
            bf_sb = [cload(t_bf[j], [P, S["fcs"][j][1]], f"bf{j}", pool=fcs)
                     for j in range(5)]
            myp = fcs.tile([g, hc4], dt, tag="myp")
            nc.gpsimd.indirect_dma_start(
                out=myp[:], out_offset=None, in_=t_pout.ap(),
                in_offset=bass.IndirectOffsetOnAxis(ap=sb_gids[:, :1],
                                                    axis=0))
            dsc = fcs.tile([g, S["desc_dim"]], dt, tag="dsc")
            nc.sync.dma_start(out=dsc[:], in_=t_desc.ap())

            def transpose_rows(z_ap, n_cols):
                out = []
                for kt in range(_ceil(n_cols, P)):
                    kn = min(P, n_cols - kt * P)
                    pst = pstr.tile([P, 512], dt, tag="tr")
                    nc.tensor.transpose(out=pst[:kn, :g],
                                        in_=z_ap[:, kt*P:kt*P+kn],
                                        identity=sb_id[:g, :g])
                    zt = fcz.tile([P, g], dt, tag="zt")
                    nc.vector.tensor_copy(out=zt[:kn, :], in_=pst[:kn, :g])
                    out.append((zt, kn))
                return out

            def fc_matmul(zt_list, w_tensor, n_out, bias_sb, relu):
                ps = fcps.tile([g, n_out], dt, tag="fps")
                r = 0
                for kt, (zt, kn) in enumerate(zt_list):
                    wt = fcw.tile([P, n_out], dt, tag="fw")
                    nc.sync.dma_start(out=wt[:kn, :],
                                      in_=w_tensor.ap()[r:r+kn, :])
                    r += kn
                    for (c0, c1) in _col_chunks(n_out):
                        nc.tensor.matmul(out=ps[:, c0:c1],
                                         lhsT=zt[:kn, :g],
                                         rhs=wt[:kn, c0:c1],
                                         start=(kt == 0),
                                         stop=(kt == len(zt_list) - 1))
                z = fcs.tile([g, n_out], dt, tag="z")
                nc.vector.tensor_tensor(out=z[:], in0=ps[:],
                                        in1=bias_sb[:g, :n_out], op=AT.add)
                if relu:
                    nc.vector.tensor_scalar_max(out=z[:], in0=z[:],
                                                scalar1=0.0)
                return z

            zt = transpose_rows(myp[:], hc4)
            zg = fc_matmul(zt, t_wg, S["wg_out"], bg_sb, relu=False)
            zt1 = transpose_rows(zg[:], S["wg_out"]) \
                + transpose_rows(dsc[:], S["desc_dim"])
            z = fc_matmul(zt1, t_wf[0], S["fcs"][0][1], bf_sb[0], relu=True)
            for j in range(1, 5):
                ztj = transpose_rows(z[:], S["fcs"][j][0])
                z = fc_matmul(ztj, t_wf[j], S["fcs"][j][1], bf_sb[j],
                              relu=(j < 4))
            nc.gpsimd.indirect_dma_start(
                out=t_out.ap(),
                out_offset=bass.IndirectOffsetOnAxis(ap=sb_gids[:, :1],
                                                     axis=0),
                in_=z[:], in_offset=None)

    nc.compile()
    return nc


# ---------------------------------------------------------------------------
# Entry point
# ---------------------------------------------------------------------------

_CACHE = {}


def kernel(**inputs):
    from concourse import bass_utils

    S, in_maps = _prep(inputs)
    key = repr(sorted(S.items()))
    if key not in _CACHE:
        _CACHE[key] = _build(S)
    nc = _CACHE[key]
    res = bass_utils.run_bass_kernel_spmd(nc, in_maps,
                                          core_ids=list(range(NCORES)))
    B, gpc = S["B"], S["gpc"]
    out = np.zeros((B, 1), np.float32)
    for c in range(NCORES):
        out[c*gpc:(c+1)*gpc] = res.results[c]["out"][c*gpc:(c+1)*gpc]
    return out


# revision 16
# speedup vs baseline: 2.1320x; 1.7475x over previous
"""Trainium2 Bass kernel for nn_GAT_Model (4x GATConv + global_add_pool + MLP).

Strategy (8 NeuronCores, SPMD single program):
  - Nodes partitioned into 8 equal ranges (1250/core, padded to 1280 local
    slots = 10 dst-blocks of 128).  Edges (incl. self-loops) are assigned to
    the core owning their dst node, sorted by dst, grouped into dst-blocks
    with a uniform tile count T_blk per block (host-padded; pad edges carry
    dstloc=-1 so their selector rows are all-zero and they contribute
    nothing).
  - Per layer: each core matmuls its own node rows producing
    [h@W | al_src | al_dst] (attention coefficients folded into the matmul
    via device-computed W@[A_s|A_d]), AllGathers the [hW | al_src] table,
    then processes its edges: dma_gather of source rows, per-edge softmax
    weights exp(leakyrelu(al_s[src]+al_d[dst])) (logits are O(1) here so no
    max subtraction is needed), aggregation of numerator and denominator via
    one-hot selector matmuls accumulating in PSUM.
  - Pooling via 256-graph selector matmuls -> partial sums -> AllReduce.
  - MLP head sharded by graph (32 graphs/core), final scatter by row ids.

kernel(**inputs) takes FULL inputs and returns the FULL [256,1] output.
"""

import math

import numpy as np

P = 128
NCORES = 8
GDT_BYTES = 2           # gather-table dtype size (2 = bf16, 4 = fp32)
CHUNK = 1024            # edges per dma_gather call
SLOTS = CHUNK // P      # gather slots (tiles of 128 edges) per chunk


# ---------------------------------------------------------------------------
# Host-side preprocessing (integer index manipulation only)
# ---------------------------------------------------------------------------

def _prep(inputs):
    x = np.asarray(inputs["x"], np.float32)
    edge_index = np.asarray(inputs["edge_index"], np.int64)
    batch = np.asarray(inputs["batch"], np.int64)
    desc = np.asarray(inputs["desc"], np.float32)

    N = x.shape[0]
    B = desc.shape[0]
    npc = N // NCORES
    assert npc * NCORES == N
    nblk = (npc + P - 1) // P
    locpad = nblk * P
    trows = NCORES * locpad
    gpc = B // NCORES
    assert gpc * NCORES == B

    convs = []
    for i in range(1, 5):
        W = np.asarray(inputs[f"W{i}"], np.float32)
        a_s = np.asarray(inputs[f"as{i}"], np.float32)
        a_d = np.asarray(inputs[f"ad{i}"], np.float32)
        b = np.asarray(inputs[f"b{i}"], np.float32)
        H, C = a_s.shape
        convs.append(dict(W=W, a_s=a_s, a_d=a_d, b=b, H=H, C=C,
                          cin=W.shape[0], hc=H * C))

    loops = np.arange(N, dtype=np.int64)
    src = np.concatenate([np.asarray(edge_index[0]), loops])
    dst = np.concatenate([np.asarray(edge_index[1]), loops])

    core_of = dst // npc
    src_tr = locpad * (src // npc) + (src % npc)   # table row per src node
    order = np.argsort(dst, kind="stable")

    blk_edges = [[[] for _ in range(nblk)] for _ in range(NCORES)]
    for e in order:
        c = int(core_of[e])
        d = int(dst[e]) - c * npc
        blk_edges[c][d // P].append((int(src_tr[e]), d - (d // P) * P))

    t_blk = max((len(bl) + P - 1) // P for cb in blk_edges for bl in cb)
    nt = nblk * t_blk
    nt_pad = ((nt + SLOTS - 1) // SLOTS) * SLOTS
    nchunks = nt_pad // SLOTS
    zrow = locpad - 1                      # pad rows are inert (dstloc=-1)

    eidx = np.full((NCORES, nt_pad * P), zrow, np.int16)
    dstloc = np.full((NCORES, nt_pad, P), -1.0, np.float32)
    for c in range(NCORES):
        for b in range(nblk):
            for j, (s_tr, d_loc) in enumerate(blk_edges[c][b]):
                g = b * t_blk + j // P
                eidx[c, g * P + (j % P)] = s_tr
                dstloc[c, g, j % P] = d_loc
    eidx_w = np.zeros((NCORES, P, nt_pad * P // 16), np.int16)
    for c in range(NCORES):
        w = eidx[c].reshape(-1, 16).T
        eidx_w[c] = np.tile(w, (8, 1))
    dstloc_t = np.ascontiguousarray(dstloc.transpose(0, 2, 1))
    dstrows = np.ascontiguousarray(dstloc)            # [NCORES, nt_pad, P]

    batchloc = np.full((NCORES, P, nblk), -1.0, np.float32)
    for c in range(NCORES):
        bl = batch[c * npc:(c + 1) * npc].astype(np.float32)
        for t in range(nblk):
            seg = bl[t * P:(t + 1) * P]
            batchloc[c, :len(seg), t] = seg

    xs = np.zeros((NCORES, locpad, x.shape[1]), np.float32)
    for c in range(NCORES):
        xs[c, :npc] = x[c * npc:(c + 1) * npc]

    amats = []
    for cv in convs:
        H, C, hc = cv["H"], cv["C"], cv["hc"]
        A = np.zeros((hc, 2 * H), np.float32)
        for h in range(H):
            A[h * C:(h + 1) * C, h] = cv["a_s"][h]
            A[h * C:(h + 1) * C, H + h] = cv["a_d"][h]
        amats.append(A)

    S = dict(
        N=N, B=B, npc=npc, nblk=nblk, locpad=locpad, trows=trows,
        gpc=gpc, ghalves=(B + P - 1) // P, t_blk=t_blk, nt=nt,
        nt_pad=nt_pad, nchunks=nchunks, desc_dim=desc.shape[1],
        xin=x.shape[1],
        convs=tuple(dict(H=c["H"], C=c["C"], cin=c["cin"], hc=c["hc"])
                    for c in convs),
        fcs=tuple((np.asarray(inputs[f"Wf{j}"]).shape[0],
                   np.asarray(inputs[f"Wf{j}"]).shape[1])
                  for j in range(1, 6)),
        wg_in=np.asarray(inputs["Wg"]).shape[0],
        wg_out=np.asarray(inputs["Wg"]).shape[1],
    )
    per256 = 256 // GDT_BYTES
    S["tcols"] = tuple(
        int(math.ceil((c["hc"] + c["H"]) * GDT_BYTES / 256) * per256)
        for c in S["convs"])
    S["gbytes"] = GDT_BYTES

    arangerow = np.tile(np.arange(256, dtype=np.float32), (P, 1))
    arangecol = np.arange(P, dtype=np.float32).reshape(P, 1)
    ident = np.eye(P, dtype=np.float32)

    in_maps = []
    for c in range(NCORES):
        m = dict(
            x=xs[c],
            eidx=eidx_w[c],
            dstloc=dstloc_t[c],
            dstrows=dstrows[c],
            batchloc=batchloc[c],
            gids=(c * gpc + np.arange(gpc, dtype=np.int32)).reshape(gpc, 1),
            desc=np.ascontiguousarray(desc[c * gpc:(c + 1) * gpc]),
            arangerow=arangerow, arangecol=arangecol, ident=ident,
            wg=np.asarray(inputs["Wg"], np.float32),
            bg=np.tile(np.asarray(inputs["bg"], np.float32), (P, 1)),
        )
        for i, cv in enumerate(convs):
            m[f"w{i+1}"] = cv["W"]
            m[f"amat{i+1}"] = amats[i]
            m[f"bias{i+1}"] = np.tile(cv["b"], (P, 1))
        for j in range(1, 6):
            m[f"wf{j}"] = np.asarray(inputs[f"Wf{j}"], np.float32)
            m[f"bf{j}"] = np.tile(np.asarray(inputs[f"bf{j}"], np.float32),
                                  (P, 1))
        in_maps.append(m)
    return S, in_maps


# ---------------------------------------------------------------------------
# Device program
# ---------------------------------------------------------------------------

def _ceil(a, b):
    return (a + b - 1) // b


def _col_chunks(n, lim=512):
    out, c = [], 0
    while c < n:
        out.append((c, min(c + lim, n)))
        c = min(c + lim, n)
    return out


def _build(S, enable_asserts=False, single=False):
    import contextlib

    import concourse.bass as bass
    import concourse.mybir as mybir
    import concourse.tile as tile
    from concourse import bacc

    dt = mybir.dt.float32
    gdt = mybir.dt.bfloat16 if S.get("gbytes", 4) == 2 else mybir.dt.float32
    nblk, t_blk, nt_pad = S["nblk"], S["t_blk"], S["nt_pad"]
    locpad, trows = S["locpad"], S["trows"]
    gpc, B = S["gpc"], S["B"]
    maxH = max(cv["H"] for cv in S["convs"])

    ndev = 1 if single else NCORES
    nc = bacc.Bacc("TRN2", target_bir_lowering=False, debug=False,
                   enable_asserts=enable_asserts, num_devices=ndev)

    # ---- I/O ----
    t_x = nc.dram_tensor("x", [locpad, S["xin"]], dt, kind="ExternalInput")
    t_eidx = nc.dram_tensor("eidx", [P, nt_pad * P // 16], mybir.dt.int16,
                            kind="ExternalInput")
    t_dstloc = nc.dram_tensor("dstloc", [P, nt_pad], dt, kind="ExternalInput")
    t_dstrows = nc.dram_tensor("dstrows", [nt_pad, P], dt,
                               kind="ExternalInput")
    t_batchloc = nc.dram_tensor("batchloc", [P, nblk], dt,
                                kind="ExternalInput")
    t_gids = nc.dram_tensor("gids", [gpc, 1], mybir.dt.int32,
                            kind="ExternalInput")
    t_desc = nc.dram_tensor("desc", [gpc, S["desc_dim"]], dt,
                            kind="ExternalInput")
    t_ar = nc.dram_tensor("arangerow", [P, 256], dt, kind="ExternalInput")
    t_ac = nc.dram_tensor("arangecol", [P, 1], dt, kind="ExternalInput")
    t_id = nc.dram_tensor("ident", [P, P], dt, kind="ExternalInput")
    t_wg = nc.dram_tensor("wg", [S["wg_in"], S["wg_out"]], dt,
                          kind="ExternalInput")
    t_bg = nc.dram_tensor("bg", [P, S["wg_out"]], dt, kind="ExternalInput")
    t_w, t_amat, t_bias, t_wf, t_bf = {}, {}, {}, {}, {}
    for i, cv in enumerate(S["convs"]):
        t_w[i] = nc.dram_tensor(f"w{i+1}", [cv["cin"], cv["hc"]], dt,
                                kind="ExternalInput")
        t_amat[i] = nc.dram_tensor(f"amat{i+1}", [cv["hc"], 2 * cv["H"]], dt,
                                   kind="ExternalInput")
        t_bias[i] = nc.dram_tensor(f"bias{i+1}", [P, cv["hc"]], dt,
                                   kind="ExternalInput")
    for j, (fi, fo) in enumerate(S["fcs"]):
        t_wf[j] = nc.dram_tensor(f"wf{j+1}", [fi, fo], dt,
                                 kind="ExternalInput")
        t_bf[j] = nc.dram_tensor(f"bf{j+1}", [P, fo], dt,
                                 kind="ExternalInput")
    t_out = nc.dram_tensor("out", [B, 1], dt, kind="ExternalOutput")

    # ---- internal DRAM ----
    tc_widths = sorted(set(S["tcols"]))
    t_bounce = {w: nc.dram_tensor(f"bounce{w}", [locpad, w], gdt,
                                  kind="Internal") for w in tc_widths}
    t_table = {w: nc.dram_tensor(f"table{w}", [trows, w], gdt,
                                 kind="Internal", addr_space="Shared")
               for w in tc_widths}
    maxin = max(cv["cin"] for cv in S["convs"])
    t_hbuf = [nc.dram_tensor(f"hbuf{k}", [locpad, maxin], dt,
                             kind="Internal") for k in range(2)]
    hc4 = S["convs"][3]["hc"]
    t_pin = nc.dram_tensor("pooled_in", [B, hc4], dt, kind="Internal")
    t_pout = nc.dram_tensor("pooled_out", [B, hc4], dt, kind="Internal",
                            addr_space="Shared")

    groups = [list(range(NCORES))]
    Exp = mybir.ActivationFunctionType.Exp
    Copy = mybir.ActivationFunctionType.Copy
    AT = mybir.AluOpType
    maxmsg = max(cv["hc"] + cv["H"] for cv in S["convs"])

    with tile.TileContext(nc) as tc, contextlib.ExitStack() as st:
        cst = st.enter_context(tc.tile_pool(name="cst", bufs=1))
        pstr = st.enter_context(tc.tile_pool(name="pstr", bufs=3,
                                             space="PSUM"))
        lyr = contextlib.ExitStack()
        wp = lyr.enter_context(tc.tile_pool(name="wp", bufs=8))
        hp = lyr.enter_context(tc.tile_pool(name="hp", bufs=3))
        htp = lyr.enter_context(tc.tile_pool(name="htp", bufs=2))
        hwxp = lyr.enter_context(tc.tile_pool(name="hwxp", bufs=2))
        selp = lyr.enter_context(tc.tile_pool(name="selp", bufs=2))
        lgp = lyr.enter_context(tc.tile_pool(name="lgp", bufs=4))
        hnp = lyr.enter_context(tc.tile_pool(name="hnp", bufs=2))
        gp = lyr.enter_context(tc.tile_pool(name="gp", bufs=2))
        fl = lyr.enter_context(tc.tile_pool(name="fl", bufs=2))

        def cload(t, shape, tag, dtt=dt, pool=None):
            s = (pool or cst).tile(shape, dtt, tag=tag, name=tag)
            nc.sync.dma_start(out=s[:], in_=t.ap())
            return s

        sb_id = cload(t_id, [P, P], "id")
        sb_ar = cload(t_ar, [P, 256], "ar")
        sb_ac = cload(t_ac, [P, 1], "ac")
        sb_eidx = cload(t_eidx, [P, nt_pad * P // 16], "eidx",
                        mybir.dt.int16)
        sb_dstloc = cload(t_dstloc, [P, nt_pad], "dstloc")
        sb_batchloc = cload(t_batchloc, [P, nblk], "batchloc")
        sb_gids = cload(t_gids, [gpc, 1], "gids", mybir.dt.int32)
        sb_bias = {i: cload(t_bias[i], [P, cv["hc"]], f"bias{i}")
                   for i, cv in enumerate(S["convs"])}
        sb_ald = cst.tile([P, nblk, maxH], gdt, tag="ald", name="ald")
        sb_was = {i: cst.tile([P, _ceil(cv["cin"], P), 2 * cv["H"]], dt,
                              tag=f"was{i}", name=f"was{i}")
                  for i, cv in enumerate(S["convs"])}
        for i in sb_was:
            nc.vector.memset(sb_was[i][:], 0)

        # ---- phase A: was_i = W_i @ [A_s | A_d] ----
        with tc.tile_pool(name="wtp", bufs=7) as wtp, \
             tc.tile_pool(name="amp", bufs=8) as amp:
            for i, cv in enumerate(S["convs"]):
                cin, hc, H = cv["cin"], cv["hc"], cv["H"]
                nk, nh = _ceil(cin, P), _ceil(hc, P)
                w_t = []
                for kt in range(nk):
                    kn = min(P, cin - kt * P)
                    wt = wp.tile([P, hc], dt, tag="w")
                    nc.sync.dma_start(out=wt[:kn, :],
                                      in_=t_w[i].ap()[kt*P:kt*P+kn, :])
                    w_t.append((wt, kn))
                wT = []
                for hcc in range(nh):
                    cw = min(P, hc - hcc * P)
                    wTt = wtp.tile([P, maxin], dt, tag="wT")
                    for kt in range(nk):
                        kn = w_t[kt][1]
                        pst = pstr.tile([P, 512], dt, tag="tr")
                        nc.tensor.transpose(
                            out=pst[:cw, :kn],
                            in_=w_t[kt][0][:kn, hcc*P:hcc*P+cw],
                            identity=sb_id[:kn, :kn])
                        nc.vector.tensor_copy(out=wTt[:cw, kt*P:kt*P+kn],
                                              in_=pst[:cw, :kn])
                    wT.append((wTt, cw))
                am = []
                for hcc in range(nh):
                    cw = wT[hcc][1]
                    amt = amp.tile([P, 2 * H], dt, tag="am")
                    nc.sync.dma_start(out=amt[:cw, :],
                                      in_=t_amat[i].ap()[hcc*P:hcc*P+cw, :])
                    am.append(amt)
                for ic in range(nk):
                    icw = min(P, cin - ic * P)
                    psw = pstr.tile([P, 512], dt, tag="tr")
                    for hcc in range(nh):
                        cw = wT[hcc][1]
                        nc.tensor.matmul(
                            out=psw[:icw, :2*H],
                            lhsT=wT[hcc][0][:cw, ic*P:ic*P+icw],
                            rhs=am[hcc][:cw, :],
                            start=(hcc == 0), stop=(hcc == nh - 1))
                    nc.vector.tensor_copy(out=sb_was[i][:icw, ic, :],
                                          in_=psw[:icw, :2*H])

        # ---- GAT layers (edge psum pools in inner scope) ----
        with tc.tile_pool(name="psacA", bufs=2, space="PSUM") as psacA, \
             tc.tile_pool(name="psacB", bufs=2, space="PSUM") as psacB:
            for i, cv in enumerate(S["convs"]):
                cin, hc, H, C = cv["cin"], cv["hc"], cv["H"], cv["C"]
                tcw = S["tcols"][i]
                nk = _ceil(cin, P)
                mmout = hc + 2 * H
                msgc = hc + H
                assert 512 < msgc <= 1024 and hc >= 512
                bounce, table = t_bounce[tcw], t_table[tcw]

                w_t = []
                for kt in range(nk):
                    kn = min(P, cin - kt * P)
                    wt = wp.tile([P, hc], dt, tag="w")
                    if kn < P:
                        z0 = (kn // 32) * 32
                        for zp in range(z0, P, 32):
                            nc.vector.memset(wt[zp:zp+32, :], 0)
                    nc.sync.dma_start(out=wt[:kn, :],
                                      in_=t_w[i].ap()[kt*P:kt*P+kn, :])
                    w_t.append((wt, kn))

                # node stage
                for t in range(nblk):
                    h_t = hp.tile([P, nk * P], dt, tag="h")
                    if cin < nk * P:
                        nc.vector.memset(h_t[:, cin:], 0)
                    if i == 0:
                        nc.sync.dma_start(out=h_t[:, :cin],
                                          in_=t_x.ap()[t*P:(t+1)*P, :])
                    else:
                        nc.sync.dma_start(
                            out=h_t[:, :cin],
                            in_=t_hbuf[(i + 1) % 2].ap()[t*P:(t+1)*P, :cin])
                    hT8 = htp.tile([P, nk * P], dt, tag="hT")
                    for grp in range(_ceil(nk, 4)):
                        gsec = min(4, nk - grp * 4)
                        pst = pstr.tile([P, 512], dt, tag="tr")
                        for j in range(gsec):
                            kt = grp * 4 + j
                            nc.tensor.transpose(out=pst[:, j*P:(j+1)*P],
                                                in_=h_t[:, kt*P:(kt+1)*P],
                                                identity=sb_id[:])
                        nc.vector.tensor_copy(
                            out=hT8[:, grp*512:grp*512+gsec*P],
                            in_=pst[:, :gsec*P])
                    pmA = psacA.tile([P, 512], dt, tag="psA")
                    pmB = psacB.tile([P, 384], dt, tag="psB")
                    for kt in range(nk):
                        stt, spp = (kt == 0), (kt == nk - 1)
                        lhs = hT8[:, kt*P:(kt+1)*P]
                        nc.tensor.matmul(out=pmA[:], lhsT=lhs,
                                         rhs=w_t[kt][0][:, :512],
                                         start=stt, stop=spp)
                        nc.tensor.matmul(out=pmB[:, :hc-512], lhsT=lhs,
                                         rhs=w_t[kt][0][:, 512:hc],
                                         start=stt, stop=False)
                        nc.tensor.matmul(out=pmB[:, hc-512:mmout-512],
                                         lhsT=lhs,
                                         rhs=sb_was[i][:, kt, :],
                                         start=False, stop=spp)
                    hwx = hwxp.tile([P, tcw], gdt, tag="hwx")
                    nc.vector.tensor_copy(out=hwx[:, :512], in_=pmA[:])
                    nc.vector.tensor_copy(out=hwx[:, 512:msgc],
                                          in_=pmB[:, :msgc-512])
                    if tcw > msgc:
                        nc.vector.memset(hwx[:, msgc:tcw], 0)
                    nc.vector.tensor_copy(out=sb_ald[:, t, :H],
                                          in_=pmB[:, msgc-512:mmout-512])
                    nc.sync.dma_start(out=bounce.ap()[t*P:(t+1)*P, :],
                                      in_=hwx[:])

                # exchange
                if single:
                    for t in range(nblk):
                        ccm = hwxp.tile([P, tcw], gdt, tag="hwx", name="ccm")
                        nc.sync.dma_start(out=ccm[:],
                                          in_=bounce.ap()[t*P:(t+1)*P, :])
                        nc.sync.dma_start(out=table.ap()[t*P:(t+1)*P, :],
                                          in_=ccm[:])
                else:
                    nc.gpsimd.collective_compute(
                        "AllGather", AT.bypass, replica_groups=groups,
                        ins=[bounce.ap()], outs=[table.ap()])

                # edge stage
                psA = psB = None
                for ch in range(S["nchunks"]):
                    g0 = ch * SLOTS
                    g_t = gp.tile([P, SLOTS, tcw], gdt, tag="g")
                    nc.gpsimd.dma_gather(
                        out_ap=g_t[:], in_ap=table.ap(),
                        idxs_ap=sb_eidx[:, ch*(CHUNK//16):(ch+1)*(CHUNK//16)],
                        num_idxs=CHUNK, num_idxs_reg=CHUNK,
                        elem_size=tcw, elem_step=tcw)
                    # batched selT: sel8[p, s, d] = (dstloc[p, g0+s] == d)
                    sel8 = selp.tile([P, SLOTS, P], gdt, tag="sel8")
                    nc.vector.tensor_tensor(
                        out=sel8[:],
                        in0=sb_dstloc[:, g0:g0+SLOTS].unsqueeze(2)
                        .to_broadcast([P, SLOTS, P]),
                        in1=sb_ar[:, :P].unsqueeze(1)
                        .to_broadcast([P, SLOTS, P]),
                        op=AT.is_equal)
                    # batched sel2: dst rows via broadcast DMA, then one
                    # all-SBUF is_equal (2x-eligible)
                    dstr8 = selp.tile([P, SLOTS, P], dt, tag="dstr8")
                    nc.sync.dma_start(
                        out=dstr8[:],
                        in_=t_dstrows.ap()[g0:g0+SLOTS, :]
                        .partition_broadcast(P))
                    sel28 = selp.tile([P, SLOTS, P], gdt, tag="sel28")
                    nc.vector.tensor_tensor(
                        out=sel28[:],
                        in0=sb_ac[:].unsqueeze(1)
                        .to_broadcast([P, SLOTS, P]),
                        in1=dstr8[:], op=AT.is_equal)
                    # batched alde
                    psal8 = pstr.tile([P, 512], dt, tag="tr")
                    for sg in range(SLOTS):
                        b = min((g0 + sg) // t_blk, nblk - 1)
                        nc.tensor.matmul(out=psal8[:, sg*H:(sg+1)*H],
                                         lhsT=sel28[:, sg, :],
                                         rhs=sb_ald[:, b, :H],
                                         start=(sg == 0),
                                         stop=(sg == SLOTS - 1))
                    # batched logits -> exp weights (into gather tile)
                    lg8 = lgp.tile([P, SLOTS, maxH], dt, tag="lg")
                    nc.vector.tensor_tensor(
                        out=lg8[:, :, :H], in0=g_t[:, :, hc:hc+H],
                        in1=psal8[:, :SLOTS*H]
                        .rearrange("p (s h) -> p s h", s=SLOTS),
                        op=AT.add)
                    lr8 = lgp.tile([P, SLOTS, maxH], dt, tag="lr")
                    nc.vector.tensor_scalar_mul(out=lr8[:, :, :H],
                                                in0=lg8[:, :, :H],
                                                scalar1=0.2)
                    nc.vector.tensor_tensor(out=lr8[:, :, :H],
                                            in0=lr8[:, :, :H],
                                            in1=lg8[:, :, :H], op=AT.max)
                    nc.scalar.activation(out=g_t[:, :, hc:hc+H],
                                         in_=lr8[:, :, :H], func=Exp)
                    # msg scaling: DVE takes slots [0:5) batched, ACT the
                    # rest per (slot, head) with per-partition scale
                    DV = SLOTS if gdt != dt else 5
                    nc.vector.tensor_tensor(
                        out=g_t[:, 0:DV, 0:hc]
                        .rearrange("p s (h c) -> p s h c", h=H),
                        in0=g_t[:, 0:DV, 0:hc]
                        .rearrange("p s (h c) -> p s h c", h=H),
                        in1=g_t[:, 0:DV, hc:hc+H].unsqueeze(3)
                        .to_broadcast([P, DV, H, C]),
                        op=AT.mult)
                    for sg in range(DV, SLOTS):
                        for h in range(H):
                            nc.scalar.activation(
                                out=g_t[:, sg, h*C:(h+1)*C],
                                in_=g_t[:, sg, h*C:(h+1)*C], func=Copy,
                                scale=g_t[:, sg, hc+h:hc+h+1])
                    for s in range(SLOTS):
                        g = ch * SLOTS + s
                        b = min(g // t_blk, nblk - 1)
                        first = (g == b * t_blk)
                        last = (g == (b + 1) * t_blk - 1) if b < nblk - 1 \
                            else (g == nt_pad - 1)
                        if first:
                            psA = psacA.tile([P, 512], dt, tag="psA")
                            psB = psacB.tile([P, 384], dt, tag="psB")
                        nc.tensor.matmul(out=psA[:], lhsT=sel8[:, s, :],
                                         rhs=g_t[:, s, :512],
                                         start=first, stop=last)
                        nc.tensor.matmul(out=psB[:, :msgc-512],
                                         lhsT=sel8[:, s, :],
                                         rhs=g_t[:, s, 512:msgc],
                                         start=first, stop=last)
                        if last:
                            def pslice(c0, c1):
                                assert (c0 >= 512) == (c1 > 512)
                                if c1 <= 512:
                                    return psA[:, c0:c1]
                                return psB[:, c0-512:c1-512]
                            sden = fl.tile([P, maxH], dt, tag="sden")
                            nc.vector.tensor_scalar_add(
                                out=sden[:, :H], in0=pslice(hc, hc + H),
                                scalar1=1e-16)
                            nc.vector.reciprocal(out=sden[:, :H],
                                                 in_=sden[:, :H])
                            hn = hnp.tile([P, hc], dt, tag="hn")
                            for h in range(H):
                                c0, c1 = h * C, (h + 1) * C
                                cuts = [c0, c1] if (c0 >= 512 or c1 <= 512) \
                                    else [c0, 512, c1]
                                for a0, a1 in zip(cuts[:-1], cuts[1:]):
                                    nc.vector.tensor_scalar(
                                        out=hn[:, a0:a1],
                                        in0=pslice(a0, a1),
                                        scalar1=sden[:, h:h+1],
                                        scalar2=None, op0=AT.mult)
                            nc.vector.tensor_tensor(
                                out=hn[:], in0=hn[:],
                                in1=sb_bias[i][:, :hc], op=AT.add)
                            nc.vector.tensor_scalar_max(
                                out=hn[:], in0=hn[:], scalar1=0.0)
                            nc.sync.dma_start(
                                out=t_hbuf[i % 2].ap()[b*P:(b+1)*P, :hc],
                                in_=hn[:])

            # ---- pooling ----
            for half in range(S["ghalves"]):
                ppA = psacA.tile([P, 512], dt, tag="psA")
                ppB = psacB.tile([P, 384], dt, tag="psB")
                for t in range(nblk):
                    h_t = hp.tile([P, hc4], dt, tag="h")
                    nc.sync.dma_start(
                        out=h_t[:], in_=t_hbuf[1].ap()[t*P:(t+1)*P, :hc4])
                    selG = selp.tile([P, P], dt, tag="selG")
                    nc.vector.tensor_tensor(
                        out=selG[:],
                        in0=sb_batchloc[:, t:t+1].to_broadcast([P, P]),
                        in1=sb_ar[:, half*P:(half+1)*P], op=AT.is_equal)
                    nc.tensor.matmul(out=ppA[:], lhsT=selG[:],
                                     rhs=h_t[:, :512],
                                     start=(t == 0), stop=(t == nblk - 1))
                    nc.tensor.matmul(out=ppB[:, :hc4-512], lhsT=selG[:],
                                     rhs=h_t[:, 512:hc4],
                                     start=(t == 0), stop=(t == nblk - 1))
                pl = fl.tile([P, hc4], dt, tag="pl")
                nc.vector.tensor_copy(out=pl[:, :512], in_=ppA[:])
                nc.vector.tensor_copy(out=pl[:, 512:hc4],
                                      in_=ppB[:, :hc4-512])
                rows = min(P, B - half * P)
                nc.sync.dma_start(out=t_pin.ap()[half*P:half*P+rows, :],
                                  in_=pl[:rows, :])
            if single:
                for half in range(S["ghalves"]):
                    ccm2 = fl.tile([P, hc4], dt, tag="pl", name="ccm2")
                    nc.sync.dma_start(out=ccm2[:],
                                      in_=t_pin.ap()[half*P:(half+1)*P, :])
                    nc.sync.dma_start(out=t_pout.ap()[half*P:(half+1)*P, :],
                                      in_=ccm2[:])
            else:
                nc.gpsimd.collective_compute(
                    "AllReduce", AT.add, replica_groups=groups,
                    ins=[t_pin.ap()], outs=[t_pout.ap()])
        lyr.close()

        # ---- FC head (layer pools closed) ----
        with tc.tile_pool(name="fcz", bufs=20) as fcz, \
             tc.tile_pool(name="fcw", bufs=3) as fcw, \
             tc.tile_pool(name="fcs", bufs=2) as fcs, \
             tc.tile_pool(name="fcps", bufs=1, space="PSUM") as fcps:
            g = gpc
            bg_sb = cload(t_bg, [P, S["wg_out"]], "bg", pool=fcs)
            bf_sb = [cload(t_bf[j], [P, S["fcs"][j][1]], f"bf{j}", pool=fcs)
                     for j in range(5)]
            myp = fcs.tile([g, hc4], dt, tag="myp")
            nc.gpsimd.indirect_dma_start(
                out=myp[:], out_offset=None, in_=t_pout.ap(),
                in_offset=bass.IndirectOffsetOnAxis(ap=sb_gids[:, :1],
                                                    axis=0))
            dsc = fcs.tile([g, S["desc_dim"]], dt, tag="dsc")
            nc.sync.dma_start(out=dsc[:], in_=t_desc.ap())

            def transpose_rows(z_ap, n_cols):
                out = []
                for kt in range(_ceil(n_cols, P)):
                    kn = min(P, n_cols - kt * P)
                    pst = pstr.tile([P, 512], dt, tag="tr")
                    nc.tensor.transpose(out=pst[:kn, :g],
                                        in_=z_ap[:, kt*P:kt*P+kn],
                                        identity=sb_id[:g, :g])
                    zt = fcz.tile([P, g], dt, tag="zt")
                    nc.vector.tensor_copy(out=zt[:kn, :], in_=pst[:kn, :g])
                    out.append((zt, kn))
                return out

            def fc_matmul(zt_list, w_tensor, n_out, bias_sb, relu):
                ps = fcps.tile([g, n_out], dt, tag="fps")
                r = 0
                for kt, (zt, kn) in enumerate(zt_list):
                    wt = fcw.tile([P, n_out], dt, tag="fw")
                    nc.sync.dma_start(out=wt[:kn, :],
                                      in_=w_tensor.ap()[r:r+kn, :])
                    r += kn
                    for (c0, c1) in _col_chunks(n_out):
                        nc.tensor.matmul(out=ps[:, c0:c1],
                                         lhsT=zt[:kn, :g],
                                         rhs=wt[:kn, c0:c1],
                                         start=(kt == 0),
                                         stop=(kt == len(zt_list) - 1))
                z = fcs.tile([g, n_out], dt, tag="z")
                nc.vector.tensor_tensor(out=z[:], in0=ps[:],
                                        in1=bias_sb[:g, :n_out], op=AT.add)
                if relu:
                    nc.vector.tensor_scalar_max(out=z[:], in0=z[:],
                                                scalar1=0.0)
                return z

            zt = transpose_rows(myp[:], hc4)
            zg = fc_matmul(zt, t_wg, S["wg_out"], bg_sb, relu=False)
            zt1 = transpose_rows(zg[:], S["wg_out"]) \
                + transpose_rows(dsc[:], S["desc_dim"])
            z = fc_matmul(zt1, t_wf[0], S["fcs"][0][1], bf_sb[0], relu=True)
            for j in range(1, 5):
                ztj = transpose_rows(z[:], S["fcs"][j][0])
                z = fc_matmul(ztj, t_wf[j], S["fcs"][j][1], bf_sb[j],
                              relu=(j < 4))
            nc.gpsimd.indirect_dma_start(
                out=t_out.ap(),
                out_offset=bass.IndirectOffsetOnAxis(ap=sb_gids[:, :1],
                                                     axis=0),
                in_=z[:], in_offset=None)

    nc.compile()
    return nc


# ---------------------------------------------------------------------------
# Entry point
# ---------------------------------------------------------------------------

_CACHE = {}


def kernel(**inputs):
    from concourse import bass_utils

    S, in_maps = _prep(inputs)
    key = repr(sorted(S.items()))
    if key not in _CACHE:
        _CACHE[key] = _build(S)
    nc = _CACHE[key]
    res = bass_utils.run_bass_kernel_spmd(nc, in_maps,
                                          core_ids=list(range(NCORES)))
    B, gpc = S["B"], S["gpc"]
    out = np.zeros((B, 1), np.float32)
    for c in range(NCORES):
        out[c*gpc:(c+1)*gpc] = res.results[c]["out"][c*gpc:(c+1)*gpc]
    return out
